# revision 1
# baseline (speedup 1.0000x reference)
"""Cross-attention (B=4, T=S=1024, C=1024, H=16, D=64) on 8 trn2 NeuronCores.

Sharding: core c handles batch b=c//2, sequence half hf=c%2 (512 q-rows).
k/v are computed for the core's own 512 encoder rows and exchanged within
the (2b, 2b+1) pair via AllGather. All activations are kept channel-major
("transposed", [C, T]-style) on chip so no transposes are ever needed; the
host transposes the per-core inputs/outputs (cheap numpy .T copies).

Per-core pipeline (everything fp32r on the PE, ~1e-4 matmul error):
  1. qT = (x Wq + bq)^T, kT likewise, v = enc Wv + bv (natural [s, c] layout,
     stored per-head padded [s, 16, 65] with a ones column at slot 64).
     RMSNorm+residual on q/k: per-token rsqrt(mean(q^2)) via ones-matmul
     column reduction + K=1 outer-product broadcast of (1 + scale*rr).
  2. Pairwise AllGather of kT [1024,512] and v_aug [512,1040].
  3. Per head h: scoresT[s-tile, t] = kh^T.T @ qh^T (K=64; head pairs run
     concurrently on PE row groups), exp on ACT (scale=1/8), then
     y_aug[65,512] = sum_s [v_h | 1].T @ exp  -- row 64 is the softmax
     denominator Z. rb = outer(1/16, 1/Z) via K=1 matmul; attn/16 = exp*rb
     accumulates into attn_mean; yT = 16 * y_aug[0:64] * rb.
  4. youtT = (yT Wp + bp)^T, DMA out; host transposes + reassembles.
"""

import numpy as np

import concourse.bacc as bacc
import concourse.mybir as mybir
import concourse.tile as tile
from concourse.bass_utils import run_bass_kernel_spmd

F32 = mybir.dt.float32
F32R = mybir.dt.float32r
AF = mybir.ActivationFunctionType
ALU = mybir.AluOpType

B, T, S, C, H = 4, 1024, 1024, 1024, 16
D = C // H            # 64
TN = 512              # per-core q rows / kv rows
KT = 8                # contraction tiles (C/128)
MT = 8                # output-channel tiles
ST = 8                # global s tiles (S/128)
DA = D + 1            # augmented head width (ones column at 64)
GROUPS = [[0, 1], [2, 3], [4, 5], [6, 7]]


def build():
    nc = bacc.Bacc("TRN2", target_bir_lowering=False, debug=False, num_devices=8)

    xT_d = nc.dram_tensor("xT", [C, TN], F32R, kind="ExternalInput")
    encT_d = nc.dram_tensor("encT", [C, TN], F32R, kind="ExternalInput")
    wq_d = nc.dram_tensor("wq", [C, C], F32R, kind="ExternalInput")
    wk_d = nc.dram_tensor("wk", [C, C], F32R, kind="ExternalInput")
    wv_d = nc.dram_tensor("wv", [C, C], F32R, kind="ExternalInput")
    wp_d = nc.dram_tensor("wp", [C, C], F32R, kind="ExternalInput")
    bq_d = nc.dram_tensor("bq", [128, MT], F32, kind="ExternalInput")
    bk_d = nc.dram_tensor("bk", [128, MT], F32, kind="ExternalInput")
    bp_d = nc.dram_tensor("bp", [128, MT], F32, kind="ExternalInput")
    bv_d = nc.dram_tensor("bv", [C], F32, kind="ExternalInput")
    qs_d = nc.dram_tensor("qs", [C], F32R, kind="ExternalInput")
    ks_d = nc.dram_tensor("ks", [C], F32R, kind="ExternalInput")

    yT_o = nc.dram_tensor("youtT", [C, TN], F32, kind="ExternalOutput")
    am_o = nc.dram_tensor("ameanT", [S, TN], F32, kind="ExternalOutput")

    KSZ = C * TN
    VSZ = TN * H * DA
    kv_bounce = nc.dram_tensor("kv_bounce", [KSZ + VSZ], F32R)
    kvg = nc.dram_tensor("kvg", [2, KSZ + VSZ], F32R)

    with tile.TileContext(nc) as tc:
        with (
            tc.tile_pool(name="const", bufs=1) as cst,
            tc.tile_pool(name="qt", bufs=1) as qt_pool,
            tc.tile_pool(name="acc", bufs=1) as acc_pool,
            tc.tile_pool(name="yt", bufs=1) as yt_pool,
        ):
            # ---- constants ----
            ones_col_f = cst.tile([128, 1], F32)
            nc.vector.memset(ones_col_f[:], 1.0)
            ones_col = cst.tile([128, 1], F32R)
            nc.vector.tensor_copy(ones_col[:], ones_col_f[:])
            inv16_row_f = cst.tile([1, 128], F32)
            nc.vector.memset(inv16_row_f[:], 1.0 / 16.0)
            inv16_row = cst.tile([1, 128], F32R)
            nc.vector.tensor_copy(inv16_row[:], inv16_row_f[:])
            ones_hf = cst.tile([128, H], F32)
            nc.vector.memset(ones_hf[:], 1.0)
            eps_t = cst.tile([1, 1], F32)
            nc.vector.memset(eps_t[:], 1e-6)

            bq_sb = cst.tile([128, MT], F32)
            bk_sb = cst.tile([128, MT], F32)
            bp_sb = cst.tile([128, MT], F32)
            nc.sync.dma_start(bq_sb[:], bq_d.ap())
            nc.sync.dma_start(bk_sb[:], bk_d.ap())
            nc.sync.dma_start(bp_sb[:], bp_d.ap())
            bv_bc = cst.tile([128, C], F32)
            nc.sync.dma_start(bv_bc[:], bv_d.ap().partition_broadcast(128))
            qs_sb = cst.tile([1, C], F32R)
            ks_sb = cst.tile([1, C], F32R)
            nc.sync.dma_start(qs_sb[:], qs_d.ap().unsqueeze(0))
            nc.sync.dma_start(ks_sb[:], ks_d.ap().unsqueeze(0))

            qT = [qt_pool.tile([128, TN], F32R, tag=f"qT{m}", name=f"qT{m}") for m in range(MT)]
            acc = [acc_pool.tile([128, TN], F32, tag=f"acc{j}", name=f"acc{j}") for j in range(ST)]
            yT = [yt_pool.tile([128, TN], F32R, tag=f"yT{k}", name=f"yT{k}") for k in range(KT)]

            # ================= phase 1: projections =================
            with (
                tc.tile_pool(name="p1_in", bufs=1) as p1_in,
                tc.tile_pool(name="p1_w", bufs=2) as p1_w,
                tc.tile_pool(name="p1_kv", bufs=1) as p1_kv,
                tc.tile_pool(name="p1_sq", bufs=2) as p1_sq,
                tc.tile_pool(name="p1_ps", bufs=2, space="PSUM") as p1_ps,
                tc.tile_pool(name="p1_ss", bufs=1, space="PSUM") as p1_ss,
                tc.tile_pool(name="p1_f", bufs=2, space="PSUM") as p1_f,
            ):
                xT = [p1_in.tile([128, TN], F32R, tag=f"xT{k}", name=f"xTs{k}") for k in range(KT)]
                eT = [p1_in.tile([128, TN], F32R, tag=f"eT{k}", name=f"eTs{k}") for k in range(KT)]
                for k in range(KT):
                    nc.sync.dma_start(xT[k][:], xT_d.ap()[k * 128 : (k + 1) * 128, :])
                    nc.sync.dma_start(eT[k][:], encT_d.ap()[k * 128 : (k + 1) * 128, :])

                def qk_projection(w_d, in_tiles, bias_sb, scale_sb, out_tiles):
                    w_sb = p1_w.tile([128, KT, C], F32R, tag="W")
                    for k in range(KT):
                        nc.sync.dma_start(
                            w_sb[:, k, :], w_d.ap()[k * 128 : (k + 1) * 128, :]
                        )
                    ssum = p1_ss.tile([1, TN], F32, tag="ssum")
                    for m in range(MT):
                        ps = p1_ps.tile([128, TN], F32, tag="proj")
                        for k in range(KT):
                            nc.tensor.matmul(
                                ps[:],
                                w_sb[:, k, m * 128 : (m + 1) * 128],
                                in_tiles[k][:],
                                start=(k == 0),
                                stop=(k == KT - 1),
                            )
                        # bias add (psum f32 -> sbuf f32r)
                        nc.vector.tensor_scalar_add(
                            out_tiles[m][:], ps[:], bias_sb[:, m : m + 1]
                        )
                        sq = p1_sq.tile([128, TN], F32R, tag="sq")
                        nc.scalar.activation(sq[:], out_tiles[m][:], AF.Square)
                        nc.tensor.matmul(
                            ssum[:],
                            ones_col[:],
                            sq[:],
                            start=(m == 0),
                            stop=(m == MT - 1),
                        )
                    # rr = 1/sqrt(ssum/C + eps)
                    rms = p1_sq.tile([1, TN], F32, tag="rms")
                    nc.scalar.activation(
                        rms[:], ssum[:], AF.Sqrt, scale=1.0 / C, bias=eps_t[:]
                    )
                    rr = p1_sq.tile([1, TN], F32R, tag="rr")
                    with nc.allow_low_precision(reason="rms rsqrt broadcast"):
                        nc.vector.reciprocal(rr[:], rms[:])
                    for m in range(MT):
                        fps = p1_f.tile([128, TN], F32, tag="fps")
                        nc.tensor.matmul(
                            fps[:],
                            scale_sb[:, m * 128 : (m + 1) * 128],
                            rr[:],
                            start=True,
                            stop=True,
                        )
                        f1 = p1_sq.tile([128, TN], F32R, tag="f1")
                        nc.scalar.activation(f1[:], fps[:], AF.Copy, bias=1.0)
                        nc.vector.tensor_mul(out_tiles[m][:], out_tiles[m][:], f1[:])

                # k projection -> kT tiles then bounce out (collective first!)
                kT = [p1_kv.tile([128, TN], F32R, tag=f"kT{m}", name=f"kTs{m}") for m in range(MT)]
                qk_projection(wk_d, eT, bk_sb, ks_sb, kT)
                for m in range(MT):
                    nc.sync.dma_start(
                        kv_bounce.ap()[m * 128 * TN : (m + 1) * 128 * TN].rearrange(
                            "(p t) -> p t", t=TN
                        ),
                        kT[m][:],
                    )

                # v projection: natural [s, c] layout, per-head padded + ones col
                v_loc = [
                    p1_kv.tile([128, H, DA], F32R, tag=f"vl{st}", name=f"vl{st}") for st in range(4)
                ]
                wv_sb = p1_w.tile([128, KT, C], F32R, tag="W")
                for k in range(KT):
                    nc.sync.dma_start(
                        wv_sb[:, k, :], wv_d.ap()[k * 128 : (k + 1) * 128, :]
                    )
                for st in range(4):
                    for jh in range(2):
                        ps = p1_ps.tile([128, TN], F32, tag="proj")
                        for k in range(KT):
                            nc.tensor.matmul(
                                ps[:],
                                eT[k][:, st * 128 : (st + 1) * 128],
                                wv_sb[:, k, jh * 512 : (jh + 1) * 512],
                                start=(k == 0),
                                stop=(k == KT - 1),
                            )
                        nc.vector.tensor_add(
                            v_loc[st][:, jh * 8 : (jh + 1) * 8, 0:D],
                            ps[:].rearrange("p (h d) -> p h d", h=8),
                            bv_bc[:, jh * 512 : (jh + 1) * 512].rearrange(
                                "p (h d) -> p h d", h=8
                            ),
                        )
                    nc.vector.tensor_copy(v_loc[st][:, :, D], ones_hf[:])
                    nc.sync.dma_start(
                        kv_bounce.ap()[
                            KSZ + st * 128 * H * DA : KSZ + (st + 1) * 128 * H * DA
                        ].rearrange("(p x) -> p x", x=H * DA),
                        v_loc[st][:].rearrange("p h d -> p (h d)"),
                    )

                # single fused collective for k+v, overlapped with q projection
                nc.gpsimd.collective_compute(
                    "AllGather",
                    ALU.bypass,
                    replica_groups=GROUPS,
                    ins=[kv_bounce.ap()],
                    outs=[kvg.ap()],
                )

                # q projection (runs on PE while the collective is in flight)
                qk_projection(wq_d, xT, bq_sb, qs_sb, qT)

            # ================= phase 3: attention =================
            with (
                tc.tile_pool(name="p3_k", bufs=1) as p3_k,
                tc.tile_pool(name="p3_v", bufs=1) as p3_v,
                tc.tile_pool(name="p3_e", bufs=2) as p3_e,
                tc.tile_pool(name="p3_rb", bufs=2) as p3_rb,
                tc.tile_pool(name="p3_sc", bufs=4, space="PSUM") as p3_sc,
                tc.tile_pool(name="p3_y", bufs=2, space="PSUM") as p3_y,
                tc.tile_pool(name="p3_rp", bufs=2, space="PSUM") as p3_rp,
            ):
                kTf = [p3_k.tile([128, 2, TN], F32R, tag=f"kTf{m}", name=f"kTf{m}") for m in range(MT)]
                for m in range(MT):
                    nc.sync.dma_start(
                        kTf[m][:],
                        kvg.ap()[:, m * 128 * TN : (m + 1) * 128 * TN]
                        .rearrange("g (p t) -> g p t", t=TN)
                        .transpose([1, 0, 2]),
                    )
                vf = [p3_v.tile([128, H, DA], F32R, tag=f"vf{j}", name=f"vf{j}") for j in range(ST)]
                for j in range(ST):
                    st = j % 4
                    nc.sync.dma_start(
                        vf[j][:],
                        kvg.ap()[
                            j // 4,
                            KSZ + st * 128 * H * DA : KSZ + (st + 1) * 128 * H * DA,
                        ].rearrange("(p h d) -> p h d", h=H, d=DA),
                    )

                for p in range(H // 2):
                    mt = p
                    heads = [(2 * p, 0), (2 * p + 1, 64)]
                    # interleaved K=64 score matmuls: bases 0/64 land on
                    # disjoint PE row groups and run concurrently
                    exps = {0: [], 64: []}
                    for j in range(ST):
                        for h, base in heads:
                            sc = p3_sc.tile([128, TN], F32, tag="sc")
                            nc.tensor.matmul(
                                sc[:],
                                kTf[mt][base : base + 64, j // 4,
                                        (j % 4) * 128 : (j % 4 + 1) * 128],
                                qT[mt][base : base + 64, :],
                                start=True,
                                stop=True,
                            )
                            ex = p3_e.tile([128, TN], F32R, tag=f"exp{base}_{j}")
                            nc.scalar.activation(
                                ex[:], sc[:], AF.Exp, scale=float(D) ** -0.5
                            )
                            exps[base].append(ex)
                    for h, base in heads:
                        y_ps = p3_y.tile([DA, TN], F32, tag="y")
                        for j in range(ST):
                            nc.tensor.matmul(
                                y_ps[:],
                                vf[j][:, h, :],
                                exps[base][j][:],
                                start=(j == 0),
                                stop=(j == ST - 1),
                            )
                        recip = p3_rb.tile([1, TN], F32R, tag="recip")
                        with nc.allow_low_precision(reason="softmax 1/Z broadcast"):
                            nc.vector.reciprocal(recip[:], y_ps[64:65, :])
                        rb_ps = p3_rp.tile([128, TN], F32, tag="rb")
                        nc.tensor.matmul(
                            rb_ps[:], inv16_row[:], recip[:], start=True, stop=True
                        )
                        # yT[c-tile mt, rows base:base+64] = 16 * y_aug * rb
                        rb_sb = p3_rb.tile([128, TN], F32, tag="rbsb")
                        nc.scalar.activation(rb_sb[:], rb_ps[:], AF.Copy)
                        nc.vector.scalar_tensor_tensor(
                            yT[mt][base : base + 64, :],
                            y_ps[0:64, :],
                            16.0,
                            rb_sb[0:64, :],
                            ALU.mult,
                            ALU.mult,
                        )
                        # attn_mean: acc[j] += exp[j] * rb  (rb = 1/(16 Z))
                        for j in range(ST):
                            if h == 0:
                                nc.vector.tensor_mul(
                                    acc[j][:], exps[base][j][:], rb_ps[:]
                                )
                            else:
                                nc.vector.tensor_mul(
                                    exps[base][j][:], exps[base][j][:], rb_ps[:]
                                )
                                nc.vector.tensor_add(
                                    acc[j][:], acc[j][:],
                                    exps[base][j][:].bitcast(F32),
                                )

            for j in range(ST):
                nc.sync.dma_start(am_o.ap()[j * 128 : (j + 1) * 128, :], acc[j][:])

            # ================= phase 4: output projection =================
            with (
                tc.tile_pool(name="p4_w", bufs=1) as p4_w,
                tc.tile_pool(name="p4_o", bufs=2) as p4_o,
                tc.tile_pool(name="p4_ps", bufs=2, space="PSUM") as p4_ps,
            ):
                wp_sb = p4_w.tile([128, KT, C], F32R, tag="Wp")
                for k in range(KT):
                    nc.sync.dma_start(
                        wp_sb[:, k, :], wp_d.ap()[k * 128 : (k + 1) * 128, :]
                    )
                for m in range(MT):
                    ps = p4_ps.tile([128, TN], F32, tag="yo")
                    for k in range(KT):
                        nc.tensor.matmul(
                            ps[:],
                            wp_sb[:, k, m * 128 : (m + 1) * 128],
                            yT[k][:],
                            start=(k == 0),
                            stop=(k == KT - 1),
                        )
                    yo = p4_o.tile([128, TN], F32, tag="yo_sb")
                    nc.vector.tensor_scalar_add(yo[:], ps[:], bp_sb[:, m : m + 1])
                    nc.sync.dma_start(yT_o.ap()[m * 128 : (m + 1) * 128, :], yo[:])

    nc.compile()
    return nc


_NC_CACHE = None


def _get_nc():
    global _NC_CACHE
    if _NC_CACHE is None:
        _NC_CACHE = build()
    return _NC_CACHE


def make_in_maps(x, encoder_output, Wq, bq, Wk, bk, Wv, bv, q_scale, k_scale,
                 Wp, bp):
    x = np.asarray(x, np.float32)
    enc = np.asarray(encoder_output, np.float32)
    Wq = np.ascontiguousarray(np.asarray(Wq, np.float32))
    Wk = np.ascontiguousarray(np.asarray(Wk, np.float32))
    Wv = np.ascontiguousarray(np.asarray(Wv, np.float32))
    Wp = np.ascontiguousarray(np.asarray(Wp, np.float32))
    bq_t = np.ascontiguousarray(np.asarray(bq, np.float32).reshape(MT, 128).T)
    bk_t = np.ascontiguousarray(np.asarray(bk, np.float32).reshape(MT, 128).T)
    bp_t = np.ascontiguousarray(np.asarray(bp, np.float32).reshape(MT, 128).T)
    bv = np.ascontiguousarray(np.asarray(bv, np.float32))
    qs = np.ascontiguousarray(np.asarray(q_scale, np.float32))
    ks = np.ascontiguousarray(np.asarray(k_scale, np.float32))

    in_maps = []
    for c in range(8):
        b, hf = c // 2, c % 2
        xT = np.ascontiguousarray(x[b, hf * TN : (hf + 1) * TN, :].T)
        encT = np.ascontiguousarray(enc[b, hf * TN : (hf + 1) * TN, :].T)
        in_maps.append(
            dict(xT=xT, encT=encT, wq=Wq, wk=Wk, wv=Wv, wp=Wp,
                 bq=bq_t, bk=bk_t, bp=bp_t, bv=bv, qs=qs, ks=ks)
        )

    return in_maps


def kernel(x, encoder_output, Wq, bq, Wk, bk, Wv, bv, q_scale, k_scale, Wp, bp,
           _trace=False):
    in_maps = make_in_maps(x, encoder_output, Wq, bq, Wk, bk, Wv, bv, q_scale,
                           k_scale, Wp, bp)
    nc = _get_nc()
    res = run_bass_kernel_spmd(nc, in_maps, core_ids=list(range(8)), trace=_trace)

    y = np.empty((B, T, C), np.float32)
    amean = np.empty((B, T, S), np.float32)
    for c in range(8):
        b, hf = c // 2, c % 2
        r = res.results[c]
        y[b, hf * TN : (hf + 1) * TN, :] = r["youtT"].T
        amean[b, hf * TN : (hf + 1) * TN, :] = r["ameanT"].T
    if _trace:
        kernel.last_exec_time_ns = res.exec_time_ns
        kernel.last_results = res
    return y, amean



# revision 2
# speedup vs baseline: 16114.7314x; 16114.7314x over previous
"""Cross-attention (B=4, T=S=1024, C=1024, H=16, D=64) on 8 trn2 NeuronCores.

Sharding: core c handles batch b=c//2, query half hf=c%2 (512 q-rows).
NO collectives: each core computes the FULL k/v for its batch (the extra
27us of PE time replaces a ~225us AllGather).  All activations stay
channel-major on chip; the host transposes per-core inputs/outputs.

All matmuls run in bf16 (PE rate identical to fp32r at these sizes, but
half the SBUF/DMA traffic); PSUM accumulation is fp32, elementwise bf16
on DVE gets the 2x packed mode.

Per-core pipeline:
  1. k = RMSNorm(enc Wk + bk)+res (full 1024 rows, two 512-col halves),
     q likewise (own 512 rows), v = enc Wv + bv stored per-head padded
     [s, 16, 65] with a ones column at slot 64.  Bias is folded into the
     PSUM->SBUF moves; sum-of-squares via ACT Square reading PSUM and a
     ones-column PE matmul; apply is one DVE scalar_tensor_tensor
     (fps+1)*raw per tile.
  2. Per head: scoresT[s,t] = kh^T.T @ qh^T (K=64, two heads of a pair on
     disjoint PE row groups), exp on ACT over 2-bank PSUM chunks -> bf16,
     y_aug[65,512] = sum_s [v|1]^T exp (row 64 = Z).  rb = outer(1/16,
     1/Z) via K=1 matmul; yT = 16*y_aug[0:64]*rb (DVE stt); attn_mean
     accB[s-tile,j,t] += exp*rb with the j-range split DVE(0:6)/Pool(6:8).
  3. youtT = (yT Wp + bp)^T -> f32 out; accB -> bf16 out (host converts).
"""

import numpy as np
import ml_dtypes

import concourse.bacc as bacc
import concourse.mybir as mybir
import concourse.tile as tile
from concourse.bass_utils import run_bass_kernel_spmd

F32 = mybir.dt.float32
F32R = mybir.dt.float32r
BF16 = mybir.dt.bfloat16
AF = mybir.ActivationFunctionType
ALU = mybir.AluOpType

B, T, S, C, H = 4, 1024, 1024, 1024, 16
D = C // H            # 64
TN = 512              # per-core q rows
KT = 8                # contraction tiles (C/128)
MT = 8                # output-channel tiles
ST = 8                # s tiles (S/128)
DA = D + 1            # augmented head width (ones column at 64)
WSPLIT = 6            # attn-mean j-tiles on DVE (rest on Pool)


def build():
    nc = bacc.Bacc("TRN2", target_bir_lowering=False, debug=False, num_devices=8)

    xT_d = nc.dram_tensor("xT", [C, TN], BF16, kind="ExternalInput")
    encT_d = nc.dram_tensor("encT", [C, S], BF16, kind="ExternalInput")
    wq_d = nc.dram_tensor("wq", [C, C], BF16, kind="ExternalInput")
    wk_d = nc.dram_tensor("wk", [C, C], BF16, kind="ExternalInput")
    wv_d = nc.dram_tensor("wv", [C, C], BF16, kind="ExternalInput")
    wp_d = nc.dram_tensor("wp", [C, C], BF16, kind="ExternalInput")
    bq_d = nc.dram_tensor("bq", [128, MT], F32, kind="ExternalInput")
    bk_d = nc.dram_tensor("bk", [128, MT], F32, kind="ExternalInput")
    bp_d = nc.dram_tensor("bp", [128, MT], F32, kind="ExternalInput")
    bv_d = nc.dram_tensor("bv", [C], F32, kind="ExternalInput")
    qs_d = nc.dram_tensor("qs", [C], F32R, kind="ExternalInput")
    ks_d = nc.dram_tensor("ks", [C], F32R, kind="ExternalInput")

    yT_o = nc.dram_tensor("youtT", [C, TN], F32, kind="ExternalOutput")
    am_o = nc.dram_tensor("ameanT", [S, TN], BF16, kind="ExternalOutput")

    with tile.TileContext(nc) as tc:
        with (
            tc.tile_pool(name="const", bufs=1) as cst,
            tc.tile_pool(name="qt", bufs=1) as qt_pool,
            tc.tile_pool(name="kt", bufs=1) as kt_pool,
            tc.tile_pool(name="vt", bufs=1) as vt_pool,
            tc.tile_pool(name="yt", bufs=1) as yt_pool,
            tc.tile_pool(name="am", bufs=1) as am_pool,
            tc.tile_pool(name="wp", bufs=1) as wp_pool,
        ):
            # ---- constants ----
            ones_col_f = cst.tile([128, 1], F32)
            nc.vector.memset(ones_col_f[:], 1.0)
            ones_col = cst.tile([128, 1], F32R)
            nc.vector.tensor_copy(ones_col[:], ones_col_f[:])
            inv16_row_f = cst.tile([1, 128], F32)
            nc.vector.memset(inv16_row_f[:], 1.0 / 16.0)
            inv16_row = cst.tile([1, 128], F32R)
            nc.vector.tensor_copy(inv16_row[:], inv16_row_f[:])
            eps_t = cst.tile([1, 1], F32)
            nc.vector.memset(eps_t[:], 1e-6)

            bq_sb = cst.tile([128, MT], F32)
            bk_sb = cst.tile([128, MT], F32)
            bp_sb = cst.tile([128, MT], F32)
            nc.sync.dma_start(bq_sb[:], bq_d.ap())
            nc.sync.dma_start(bk_sb[:], bk_d.ap())
            nc.sync.dma_start(bp_sb[:], bp_d.ap())
            bv_bc = cst.tile([128, C], F32)
            nc.sync.dma_start(bv_bc[:], bv_d.ap().partition_broadcast(128))
            qs_sb = cst.tile([1, C], F32R)
            ks_sb = cst.tile([1, C], F32R)
            nc.sync.dma_start(qs_sb[:], qs_d.ap().unsqueeze(0))
            nc.sync.dma_start(ks_sb[:], ks_d.ap().unsqueeze(0))

            qT = [qt_pool.tile([128, TN], BF16, tag=f"qT{m}", name=f"qT{m}")
                  for m in range(MT)]
            kT = [kt_pool.tile([128, 2, TN], BF16, tag=f"kT{m}", name=f"kT{m}")
                  for m in range(MT)]
            vA = [vt_pool.tile([128, H, DA], BF16, tag=f"vA{j}", name=f"vA{j}")
                  for j in range(ST)]
            yT = [yt_pool.tile([128, TN], BF16, tag=f"yT{k}", name=f"yT{k}")
                  for k in range(KT)]
            accB = am_pool.tile([128, ST, TN], BF16, name="accB")
            wp_sb = wp_pool.tile([128, KT, C], BF16, name="wp_sb")

            # ================= phase 1: projections =================
            with (
                tc.tile_pool(name="p1_in", bufs=1) as p1_in,
                tc.tile_pool(name="p1_w", bufs=2) as p1_w,
                tc.tile_pool(name="p1_raw", bufs=2) as p1_raw,
                tc.tile_pool(name="p1_sq", bufs=2) as p1_sq,
                tc.tile_pool(name="p1_ps", bufs=3, space="PSUM") as p1_ps,
                tc.tile_pool(name="p1_ss", bufs=2, space="PSUM") as p1_ss,
                tc.tile_pool(name="p1_f", bufs=2, space="PSUM") as p1_f,
            ):
                # input + weight DMAs (emission order = SP issue order)
                wk_sb = p1_w.tile([128, KT, C], BF16, tag="W", name="wk_sb")
                nc.sync.dma_start(
                    wk_sb[:], wk_d.ap().rearrange("(k p) c -> p k c", p=128)
                )
                e_sb = p1_in.tile([128, KT, 2, TN], BF16, name="e_sb")
                nc.sync.dma_start(
                    e_sb[:],
                    encT_d.ap().rearrange("(k p) (h t) -> p k h t", p=128, h=2),
                )
                wq_sb = p1_w.tile([128, KT, C], BF16, tag="W", name="wq_sb")
                nc.sync.dma_start(
                    wq_sb[:], wq_d.ap().rearrange("(k p) c -> p k c", p=128)
                )
                x_sb = p1_in.tile([128, KT, TN], BF16, name="x_sb")
                nc.sync.dma_start(
                    x_sb[:], xT_d.ap().rearrange("(k p) t -> p k t", p=128)
                )
                wv_sb = p1_w.tile([128, KT, C], BF16, tag="W", name="wv_sb")
                nc.sync.dma_start(
                    wv_sb[:], wv_d.ap().rearrange("(k p) c -> p k c", p=128)
                )
                nc.sync.dma_start(
                    wp_sb[:], wp_d.ap().rearrange("(k p) c -> p k c", p=128)
                )

                def qk_proj(w_sb, in_ap, bias_sb, scale_sb, out_ap):
                    """One 512-column projection + RMSNorm+residual.

                    in_ap(k) -> [128, TN] bf16; out_ap(m) -> [128, TN] bf16.
                    """
                    ssum = p1_ss.tile([1, TN], F32, tag="ssum")
                    raws = []
                    for m in range(MT):
                        ps = p1_ps.tile([128, TN], F32, tag="proj")
                        for k in range(KT):
                            nc.tensor.matmul(
                                ps[:],
                                w_sb[:, k, m * 128 : (m + 1) * 128],
                                in_ap(k),
                                start=(k == 0),
                                stop=(k == KT - 1),
                            )
                        raw = p1_raw.tile([128, TN], F32R, tag=f"raw{m}")
                        nc.vector.tensor_scalar_add(
                            raw[:], ps[:], bias_sb[:, m : m + 1]
                        )
                        sq = p1_sq.tile([128, TN], F32R, tag="sq")
                        nc.scalar.activation(
                            sq[:], ps[:], AF.Square, bias=bias_sb[:, m : m + 1]
                        )
                        nc.tensor.matmul(
                            ssum[:], ones_col[:], sq[:],
                            start=(m == 0), stop=(m == MT - 1),
                        )
                        raws.append(raw)
                    rms = p1_sq.tile([1, TN], F32, tag="rms")
                    nc.scalar.activation(
                        rms[:], ssum[:], AF.Sqrt, scale=1.0 / C, bias=eps_t[:]
                    )
                    rr = p1_sq.tile([1, TN], F32R, tag="rr")
                    with nc.allow_low_precision(reason="rms rsqrt broadcast"):
                        nc.vector.reciprocal(rr[:], rms[:])
                    for m in range(MT):
                        fps = p1_f.tile([128, TN], F32, tag="fps")
                        nc.tensor.matmul(
                            fps[:],
                            scale_sb[0:1, m * 128 : (m + 1) * 128],
                            rr[:],
                            start=True,
                            stop=True,
                        )
                        # out = (fps + 1) * raw   (rmsnorm + residual)
                        nc.vector.scalar_tensor_tensor(
                            out_ap(m), fps[:], 1.0, raws[m][:],
                            ALU.add, ALU.mult,
                        )

                for hf2 in range(2):
                    qk_proj(
                        wk_sb,
                        lambda k, hf2=hf2: e_sb[:, k, hf2, :],
                        bk_sb,
                        ks_sb,
                        lambda m, hf2=hf2: kT[m][:, hf2, :],
                    )
                qk_proj(
                    wq_sb,
                    lambda k: x_sb[:, k, :],
                    bq_sb,
                    qs_sb,
                    lambda m: qT[m][:, :],
                )

                # v projection: natural [s, c] layout, per-head padded
                for st in range(ST):
                    nc.gpsimd.memset(vA[st][:, :, D:DA], 1.0)
                    for jh in range(2):
                        ps = p1_ps.tile([128, TN], F32, tag="proj")
                        for k in range(KT):
                            nc.tensor.matmul(
                                ps[:],
                                e_sb[:, k, st // 4, (st % 4) * 128 : (st % 4 + 1) * 128],
                                wv_sb[:, k, jh * 512 : (jh + 1) * 512],
                                start=(k == 0),
                                stop=(k == KT - 1),
                            )
                        nc.vector.tensor_add(
                            vA[st][:, jh * 8 : (jh + 1) * 8, 0:D],
                            ps[:].rearrange("p (h d) -> p h d", h=8),
                            bv_bc[:, jh * 512 : (jh + 1) * 512].rearrange(
                                "p (h d) -> p h d", h=8
                            ),
                        )

            # ================= phase 2: attention =================
            with (
                tc.tile_pool(name="p2_e", bufs=2) as p2_e,
                tc.tile_pool(name="p2_rb", bufs=4) as p2_rb,
                tc.tile_pool(name="p2_sc", bufs=2, space="PSUM") as p2_sc,
                tc.tile_pool(name="p2_y", bufs=2, space="PSUM") as p2_y,
                tc.tile_pool(name="p2_rp", bufs=2, space="PSUM") as p2_rp,
            ):
                for p in range(H // 2):
                    mt = p
                    expT = p2_e.tile([128, 2, ST, TN], BF16, tag="exp")
                    for hh in range(2):
                        h = 2 * p + hh
                        base = 64 * hh
                        for jj2 in range(4):
                            sc = p2_sc.tile([128, 2, TN], F32, tag="sc")
                            for u in range(2):
                                j = 2 * jj2 + u
                                nc.tensor.matmul(
                                    sc[:, u, :],
                                    kT[mt][base : base + 64, j // 4,
                                           (j % 4) * 128 : (j % 4 + 1) * 128],
                                    qT[mt][base : base + 64, :],
                                    start=True,
                                    stop=True,
                                )
                            nc.scalar.activation(
                                expT[:, hh, 2 * jj2 : 2 * jj2 + 2, :],
                                sc[:],
                                AF.Exp,
                                scale=float(D) ** -0.5,
                            )
                        y_ps = p2_y.tile([DA, TN], F32, tag="y")
                        for j in range(ST):
                            nc.tensor.matmul(
                                y_ps[:],
                                vA[j][:, h, :],
                                expT[:, hh, j, :],
                                start=(j == 0),
                                stop=(j == ST - 1),
                            )
                        recip = p2_rb.tile([1, TN], F32R, tag="recip")
                        with nc.allow_low_precision(reason="softmax 1/Z"):
                            nc.vector.reciprocal(recip[:], y_ps[64:65, :])
                        rb_ps = p2_rp.tile([128, TN], F32, tag="rb")
                        nc.tensor.matmul(
                            rb_ps[:], inv16_row[:], recip[:], start=True, stop=True
                        )
                        rb_sb = p2_rb.tile([128, TN], BF16, tag="rbsb")
                        nc.scalar.activation(rb_sb[:], rb_ps[:], AF.Copy)
                        # yT[c-tile mt, rows base:base+64] = 16 * y_aug * rb
                        nc.vector.scalar_tensor_tensor(
                            yT[mt][base : base + 64, :],
                            y_ps[0:64, :],
                            16.0,
                            rb_sb[0:64, :],
                            ALU.mult,
                            ALU.mult,
                        )
                        # attn_mean: accB[:, j, :] += exp_h[:, j, :] * rb
                        rbb = rb_sb[:].rearrange("p (a t) -> p a t", a=1)
                        wd, wp_ = WSPLIT, ST - WSPLIT
                        if h == 0:
                            nc.vector.tensor_mul(
                                accB[:, 0:wd, :], expT[:, hh, 0:wd, :],
                                rbb.broadcast_to((128, wd, TN)),
                            )
                            nc.gpsimd.tensor_mul(
                                accB[:, wd:ST, :], expT[:, hh, wd:ST, :],
                                rbb.broadcast_to((128, wp_, TN)),
                            )
                        else:
                            nc.vector.tensor_mul(
                                expT[:, hh, 0:wd, :], expT[:, hh, 0:wd, :],
                                rbb.broadcast_to((128, wd, TN)),
                            )
                            nc.vector.tensor_add(
                                accB[:, 0:wd, :], accB[:, 0:wd, :],
                                expT[:, hh, 0:wd, :],
                            )
                            nc.gpsimd.tensor_mul(
                                expT[:, hh, wd:ST, :], expT[:, hh, wd:ST, :],
                                rbb.broadcast_to((128, wp_, TN)),
                            )
                            nc.gpsimd.tensor_add(
                                accB[:, wd:ST, :], accB[:, wd:ST, :],
                                expT[:, hh, wd:ST, :],
                            )

            nc.sync.dma_start(
                am_o.ap().rearrange("(j p) t -> p j t", p=128), accB[:]
            )

            # ================= phase 3: output projection =================
            with (
                tc.tile_pool(name="p3_o", bufs=2) as p3_o,
                tc.tile_pool(name="p3_ps", bufs=2, space="PSUM") as p3_ps,
            ):
                for m in range(MT):
                    ps = p3_ps.tile([128, TN], F32, tag="yo")
                    for k in range(KT):
                        nc.tensor.matmul(
                            ps[:],
                            wp_sb[:, k, m * 128 : (m + 1) * 128],
                            yT[k][:],
                            start=(k == 0),
                            stop=(k == KT - 1),
                        )
                    yo = p3_o.tile([128, TN], F32, tag="yo_sb")
                    nc.vector.tensor_scalar_add(yo[:], ps[:], bp_sb[:, m : m + 1])
                    nc.sync.dma_start(yT_o.ap()[m * 128 : (m + 1) * 128, :], yo[:])

    nc.compile()
    return nc


_NC_CACHE = None


def _get_nc():
    global _NC_CACHE
    if _NC_CACHE is None:
        _NC_CACHE = build()
    return _NC_CACHE


BF = ml_dtypes.bfloat16


def make_in_maps(x, encoder_output, Wq, bq, Wk, bk, Wv, bv, q_scale, k_scale,
                 Wp, bp):
    x = np.asarray(x, np.float32)
    enc = np.asarray(encoder_output, np.float32)
    Wq_b = np.ascontiguousarray(np.asarray(Wq, np.float32)).astype(BF)
    Wk_b = np.ascontiguousarray(np.asarray(Wk, np.float32)).astype(BF)
    Wv_b = np.ascontiguousarray(np.asarray(Wv, np.float32)).astype(BF)
    Wp_b = np.ascontiguousarray(np.asarray(Wp, np.float32)).astype(BF)
    bq_t = np.ascontiguousarray(np.asarray(bq, np.float32).reshape(MT, 128).T)
    bk_t = np.ascontiguousarray(np.asarray(bk, np.float32).reshape(MT, 128).T)
    bp_t = np.ascontiguousarray(np.asarray(bp, np.float32).reshape(MT, 128).T)
    bv = np.ascontiguousarray(np.asarray(bv, np.float32))
    qs = np.ascontiguousarray(np.asarray(q_scale, np.float32))
    ks = np.ascontiguousarray(np.asarray(k_scale, np.float32))

    encT_b = [np.ascontiguousarray(enc[b].T).astype(BF) for b in range(B)]

    in_maps = []
    for c in range(8):
        b, hf = c // 2, c % 2
        xT = np.ascontiguousarray(x[b, hf * TN : (hf + 1) * TN, :].T).astype(BF)
        in_maps.append(
            dict(xT=xT, encT=encT_b[b], wq=Wq_b, wk=Wk_b, wv=Wv_b, wp=Wp_b,
                 bq=bq_t, bk=bk_t, bp=bp_t, bv=bv, qs=qs, ks=ks)
        )
    return in_maps


def kernel(x, encoder_output, Wq, bq, Wk, bk, Wv, bv, q_scale, k_scale, Wp, bp,
           _trace=False):
    in_maps = make_in_maps(x, encoder_output, Wq, bq, Wk, bk, Wv, bv, q_scale,
                           k_scale, Wp, bp)
    nc = _get_nc()
    res = run_bass_kernel_spmd(nc, in_maps, core_ids=list(range(8)), trace=_trace)

    y = np.empty((B, T, C), np.float32)
    amean = np.empty((B, T, S), np.float32)
    for c in range(8):
        b, hf = c // 2, c % 2
        r = res.results[c]
        y[b, hf * TN : (hf + 1) * TN, :] = r["youtT"].T
        amean[b, hf * TN : (hf + 1) * TN, :] = (
            r["ameanT"].astype(np.float32).T
        )
    if _trace:
        kernel.last_exec_time_ns = res.exec_time_ns
        kernel.last_results = res
    return y, amean
